# revision 4
# baseline (speedup 1.0000x reference)
"""MoE gating kernel for Trainium2 (Bass/Tile), data-parallel over 8 NeuronCores.

Computes: logits = x @ W_g.T ; top-2 values; softmax over the 2 values.
  p1 = sigmoid(v1 - v2), p2 = sigmoid(v2 - v1)  (v1 >= v2 the top-2 logits)

Sharding: tokens split 8 ways (2048 tokens/core), W_g replicated.

v2 design (from the 73.2us baseline):
  - The x stream is engine-bound: the 16 SDMA engines process one 8KB fp32
    row per ~306ns (~27 GB/s each) regardless of DGE path, so 2048 rows
    take ~39.2us spread across engines. The baseline lost ~9us waiting for
    the SWDGE Q7 boot (first descriptor ~9.4us) and ~10us to the
    engine-7/15 SWDGE descriptor-ring straggler effect.
  - Mixed stream: most tiles go as fp32 via HWDGE (sync+scalar rings,
    dispatch from ~6.5us, engines balanced) and are cast fp32->bf16
    on-chip (split DVE/ACT, GpSimd joining once its Q7 finishes emitting
    SWDGE descriptors); a few tiles keep the SWDGE converting path so the
    Q7/descriptor bandwidth still contributes and the cast load stays
    bounded.
  - bf16 transposes via regular matmul against identity (HAM-visible),
    G=512 grouped main matmuls (N=512 moving amortizes LDWEIGHTS), last
    group split in half to shrink the serial tail.
  - Sigmoid ACT table preloaded early (dummy sigmoid after the first ACT
    copy) so the tail doesn't pay the 1.28us ACT_TABLE_LOAD.
  - One contiguous partition-major store; host de-interleaves.
"""

import sys

sys.path.insert(0, "/opt/trn_rl_repo")

from contextlib import ExitStack

import numpy as np
import ml_dtypes

import concourse.bass as bass
import concourse.bacc as bacc
import concourse.mybir as mybir
from concourse.tile import TileContext
from concourse.bass_utils import run_bass_kernel_spmd

TOKENS = 16384
DIM = 2048
E = 64  # num experts
NCORES = 8
TPC = TOKENS // NCORES  # tokens per core
P = 128
KT = DIM // P  # 16 contraction tiles
NB = TPC // P  # 16 token blocks (tiles) per core
G = 512  # token group for the main matmul
TPG = G // P  # 4 tiles per group
NG = TPC // G  # 4 groups

F32 = mybir.dt.float32
BF16 = mybir.dt.bfloat16
N_WARM = 20

# tiles arriving via SWDGE converting DMA (bf16 direct); rest HWDGE fp32
S_TILES = frozenset({2, 4, 6, 8, 10, 12})
# HWDGE ring per H tile: listed ones ride sync, others scalar
SYNC_RING = frozenset({0, 3, 7, 11, 14})
# H tiles whose cast GpSimd helps with (its Q7 is busy emitting SWDGE
# descriptors until ~20us, so early tiles split DVE/ACT only)
GP_CAST = frozenset({7, 9, 11, 13, 14, 15})


def _emit(tc, ctx, x_ap, wgt_ap, idb_ap, idf_ap, out_ap):
    nc = tc.nc

    singles = ctx.enter_context(tc.tile_pool(name="singles", bufs=1))
    psum_t = ctx.enter_context(tc.tile_pool(name="psum_t", bufs=3, space="PSUM"))
    psum_l = ctx.enter_context(tc.tile_pool(name="psum_l", bufs=2, space="PSUM"))
    psum_f = ctx.enter_context(tc.tile_pool(name="psum_f", bufs=1, space="PSUM"))
    psum_w = ctx.enter_context(tc.tile_pool(name="psum_w", bufs=1, space="PSUM"))
    ltpool = ctx.enter_context(tc.tile_pool(name="ltpool", bufs=2))
    spool = ctx.enter_context(tc.tile_pool(name="spool", bufs=4))
    xt_pool = ctx.enter_context(tc.tile_pool(name="xt", bufs=2))

    warm = singles.tile([P, P], BF16)
    warm_rhs = singles.tile([P, 4 * P], BF16)
    nc.vector.memset(warm[:], 0.0)
    nc.vector.memset(warm_rhs[:], 0.0)

    warm_flip = [False]

    def warm_mm():
        # alternate PSUM banks so back-to-back warm-ups don't serialize on
        # the write-after-write; sustained duty is what flips HAM
        warm_flip[0] = not warm_flip[0]
        if warm_flip[0]:
            pw = psum_w.tile([P, 4 * P], F32, tag="warm_ps")
        else:
            pw = psum_f.tile([P, 4 * P], F32, tag="fin_ps")
        nc.tensor.matmul(pw[:], warm[:], warm_rhs[:])

    for _ in range(N_WARM):
        warm_mm()

    # x tile DMAs: HWDGE fp32 on sync/scalar rings, SWDGE converting for
    # S_TILES. All dma_starts emitted up-front so each queue streams freely.
    xf32 = {}
    xb = [
        singles.tile([P, DIM], BF16, tag=f"xb{t}", name=f"xb{t}") for t in range(NB)
    ]

    ident = singles.tile([P, P], BF16)
    ident_f = singles.tile([P, P], F32)
    wgT = singles.tile([P, KT, E], BF16)

    sync_emitted = 0
    for t in range(NB):
        src = x_ap[t * P : (t + 1) * P, :]
        if t in S_TILES:
            nc.gpsimd.dma_start(out=xb[t][:], in_=src)
        else:
            xf = singles.tile([P, DIM], F32, tag=f"xf{t}")
            xf32[t] = xf
            eng = nc.sync if t in SYNC_RING else nc.scalar
            eng.dma_start(out=xf[:], in_=src)
            if eng is nc.sync:
                sync_emitted += 1
                if sync_emitted == 2:
                    # constants ride the sync ring after the first two x
                    # tiles so the stream head isn't delayed
                    nc.sync.dma_start(out=ident[:], in_=idb_ap)
                    nc.sync.dma_start(out=ident_f[:], in_=idf_ap)
                    nc.sync.dma_start(out=wgT[:], in_=wgt_ap)

    # per-token-block (v1-v2, v2-v1) accumulate here; one sigmoid + one
    # contiguous partition-major store at the end
    dd_all = singles.tile([P, NB, 2], F32)
    sig_scratch = singles.tile([1, 2], F32)
    sig_preloaded = [False]

    def cast_tile(t):
        # fp32 -> bf16 token-major, split across engines
        xf = xf32[t]
        if t in GP_CAST:
            nc.vector.tensor_copy(xb[t][:, 0:512], xf[:, 0:512])
            nc.scalar.copy(xb[t][:, 512:1280], xf[:, 512:1280])
            nc.gpsimd.tensor_copy(xb[t][:, 1280:DIM], xf[:, 1280:DIM])
        else:
            nc.vector.tensor_copy(xb[t][:, 0:1024], xf[:, 0:1024])
            nc.scalar.copy(xb[t][:, 1024:DIM], xf[:, 1024:DIM])
        if not sig_preloaded[0]:
            # preload the sigmoid ACT table while the stream runs
            sig_preloaded[0] = True
            nc.scalar.activation(
                sig_scratch[:], ident_f[0:1, 0:2], mybir.ActivationFunctionType.Sigmoid
            )

    def transpose_tile(t, xt_g):
        # 16 regular bf16 matmuls vs identity; xt_g layout [P, tb, k, 128t]
        # so drains are contiguous and the main matmul reads k-slices strided
        tb = t % TPG
        for q in range(KT // 4):
            pt = psum_t.tile([P, 4 * P], F32)
            for j in range(4):
                k = 4 * q + j
                nc.tensor.matmul(
                    pt[:, j * P : (j + 1) * P],
                    xb[t][:, k * P : (k + 1) * P],
                    ident[:],
                )
            dst = xt_g[:, tb, 4 * q : 4 * q + 4, :]
            if q % 2 == 0:
                nc.vector.tensor_copy(dst, pt[:])
            else:
                nc.scalar.copy(dst, pt[:])

    def epilogue(lp, width, tiles):
        # back to token-major + top-2 for `width` tokens; `tiles` are the
        # global tile indices covered
        lt = ltpool.tile([E, G], F32, tag="lt")
        nc.vector.tensor_copy(lt[:, 0:width], lp[:, 0:width])
        for i, t in enumerate(tiles):
            c = i * P
            fp = psum_f.tile([P, E], F32, tag="fin_ps")
            nc.tensor.matmul(
                fp[:], lt[:, c : c + P], ident_f[:E, :E], is_transpose=True
            )
            max8 = spool.tile([P, 8], F32)
            nc.vector.max(out=max8[:], in_=fp[:])
            nc.vector.tensor_sub(dd_all[:, t, 0:1], max8[:, 0:1], max8[:, 1:2])
            nc.vector.tensor_sub(dd_all[:, t, 1:2], max8[:, 1:2], max8[:, 0:1])

    for g in range(NG):
        xt_g = xt_pool.tile([P, TPG, KT, P], BF16)
        for tb in range(TPG):
            t = g * TPG + tb
            if t not in S_TILES:
                cast_tile(t)
            transpose_tile(t, xt_g)
            if tb % 2 == 1:
                warm_mm()  # keeper: bridge PE data-waits below HAM's MID window
        if g < NG - 1:
            lp = psum_l.tile([E, G], F32)
            for k in range(KT):
                nc.tensor.matmul(
                    lp[:],
                    wgT[:, k, :],
                    xt_g[:, :, k, :],
                    start=(k == 0),
                    stop=(k == KT - 1),
                )
            epilogue(lp, G, [g * TPG + i for i in range(TPG)])
        else:
            # final group: two half-width mains so the tail after the last
            # byte is one half-group, not a whole one
            for h in range(2):
                lp = psum_l.tile([E, 2 * P], F32)
                for k in range(KT):
                    nc.tensor.matmul(
                        lp[:],
                        wgT[:, k, :],
                        xt_g[:, 2 * h : 2 * h + 2, k, :],
                        start=(k == 0),
                        stop=(k == KT - 1),
                    )
                epilogue(lp, 2 * P, [g * TPG + 2 * h + i for i in range(2)])

    # single sigmoid + one contiguous partition-major store
    ot = singles.tile([P, NB, 2], F32)
    nc.scalar.activation(ot[:], dd_all[:], mybir.ActivationFunctionType.Sigmoid)
    nc.sync.dma_start(out=out_ap, in_=ot[:])


_NC_CACHE = {}


def _build():
    key = "nc"
    if key in _NC_CACHE:
        return _NC_CACHE[key]
    nc = bacc.Bacc(trn_type="TRN2")
    x = nc.dram_tensor("x", [TPC, DIM], F32, kind="ExternalInput")
    wgt = nc.dram_tensor("wgt", [P, KT * E], BF16, kind="ExternalInput")
    idb = nc.dram_tensor("idb", [P, P], BF16, kind="ExternalInput")
    idf = nc.dram_tensor("idf", [P, P], F32, kind="ExternalInput")
    out = nc.dram_tensor("out", [P, NB * 2], F32, kind="ExternalOutput")
    with TileContext(nc) as tc, ExitStack() as ctx:
        _emit(tc, ctx, x.ap(), wgt.ap(), idb.ap(), idf.ap(), out.ap())
    if not nc.is_finalized():
        nc.finalize()
    _NC_CACHE[key] = nc
    return nc


def _run(x, W_g, trace=False):
    nc = _build()
    x = np.ascontiguousarray(np.asarray(x, dtype=np.float32))
    W_g = np.asarray(W_g, dtype=np.float32)
    # host-side weight layout prep: wgt[p, k*E + e] = W_g[e, k*128 + p]
    wgt = np.ascontiguousarray(
        W_g.reshape(E, KT, P).transpose(2, 1, 0).reshape(P, KT * E)
    ).astype(ml_dtypes.bfloat16)
    idb = np.eye(P, dtype=np.float32).astype(ml_dtypes.bfloat16)
    idf = np.eye(P, dtype=np.float32)
    in_maps = [
        {
            "x": np.ascontiguousarray(x[c * TPC : (c + 1) * TPC]),
            "wgt": wgt,
            "idb": idb,
            "idf": idf,
        }
        for c in range(NCORES)
    ]
    res = run_bass_kernel_spmd(nc, in_maps, core_ids=list(range(NCORES)), trace=trace)
    # device output is partition-major [128, 16, 2]; de-interleave:
    # out[b*128 + p, :] = res[p, b, :]
    outs = []
    for r in res.results:
        o = r["out"].reshape(P, NB, 2).transpose(1, 0, 2).reshape(TPC, 2)
        outs.append(o)
    out = np.ascontiguousarray(np.concatenate(outs, axis=0))
    return out, res


def kernel(x, W_g):
    out, _ = _run(x, W_g, trace=False)
    return out


def kernel_profiled(x, W_g, **_kw):
    out, res = _run(x, W_g, trace=True)
    return out, res


# revision 6
# speedup vs baseline: 1.0446x; 1.0446x over previous
"""MoE gating kernel for Trainium2 (Bass/Tile), data-parallel over 8 NeuronCores.

Computes: logits = x @ W_g.T ; top-2 values; softmax over the 2 values.
  p1 = sigmoid(v1 - v2), p2 = sigmoid(v2 - v1)  (v1 >= v2 the top-2 logits)

Sharding: tokens split 8 ways (2048 tokens/core), W_g replicated.

v3 design (from the 73.2us baseline):
  - The x stream is engine-bound: the 16 SDMA engines process one 8KB fp32
    row per ~306ns (~27 GB/s each) regardless of DGE path; 2048 rows take
    ~39.2us. The baseline lost ~9us to the SWDGE Q7 boot and ~10us to the
    engine-7/15 SWDGE straggler.
  - 12 tiles ride HWDGE fp32 (sync+scalar rings, dispatch ~6.8us, engines
    balanced); 4 tiles keep SWDGE converting DMA (Q7 emission fits before
    its cast duties). Constants ride SWDGE so they don't consume DMAHW
    completion lanes.
  - Tile's scheduler has only 8 DMAHW completion-sem lanes for HWDGE DMAs;
    the 9th+ DMA waits on its lane predecessor AT DISPATCH, blocking that
    sequencer's FIFO (this serialized a previous attempt to 105us). So
    exactly 8 HWDGE x DMAs dispatch up-front; the last 4 are emitted
    interleaved into the compute stream after their lane predecessors are
    long complete.
  - HWDGE fp32 tiles are cast fp32->bf16 on-chip, split DVE/ACT (+GpSimd
    512-col share on late tiles; its Q7 is busy until ~16us and only
    manages ~34 Gelem/s).
  - bf16 transposes via regular matmul against identity (HAM-visible),
    G=512 grouped main matmuls; the last group runs per-pair/per-tile
    mains so the serial tail after the last byte is one tile, not a group.
  - Sigmoid ACT table preloaded early (dummy sigmoid after the first ACT
    cast) so the tail doesn't pay the 1.28us ACT_TABLE_LOAD.
  - One contiguous partition-major store; host de-interleaves.
"""

import sys

sys.path.insert(0, "/opt/trn_rl_repo")

from contextlib import ExitStack

import numpy as np
import ml_dtypes

import concourse.bass as bass
import concourse.bacc as bacc
import concourse.mybir as mybir
from concourse.tile import TileContext
from concourse.bass_utils import run_bass_kernel_spmd

TOKENS = 16384
DIM = 2048
E = 64  # num experts
NCORES = 8
TPC = TOKENS // NCORES  # tokens per core
P = 128
KT = DIM // P  # 16 contraction tiles
NB = TPC // P  # 16 token blocks (tiles) per core
G = 512  # token group for the main matmul
TPG = G // P  # 4 tiles per group
NG = TPC // G  # 4 groups

F32 = mybir.dt.float32
BF16 = mybir.dt.bfloat16
N_WARM = 20

S_TILES = (2, 5, 8, 11)  # SWDGE converting tiles (mid-stream)
SYNC_TILES = (0, 3, 6, 9, 12, 14)  # HWDGE sync ring, in ring order
ACT_TILES = (1, 4, 7, 10, 13, 15)  # HWDGE scalar ring, in ring order
# the last two DMAs on each ring exceed the 8 up-front lanes; they are
# emitted later, keyed by "emit after tile T's compute emission"
LATE_EMIT = {12: 3, 14: 5, 13: 4, 15: 6}
# tiles whose cast GpSimd helps with ([128, 512] tail share)
GP_CAST = frozenset({6, 7, 9, 10, 12, 13, 14, 15})


def _emit(tc, ctx, x_ap, wgt_ap, idb_ap, idf_ap, out_ap):
    nc = tc.nc

    singles = ctx.enter_context(tc.tile_pool(name="singles", bufs=1))
    psum_t = ctx.enter_context(tc.tile_pool(name="psum_t", bufs=3, space="PSUM"))
    psum_l = ctx.enter_context(tc.tile_pool(name="psum_l", bufs=2, space="PSUM"))
    psum_f = ctx.enter_context(tc.tile_pool(name="psum_f", bufs=1, space="PSUM"))
    psum_w = ctx.enter_context(tc.tile_pool(name="psum_w", bufs=1, space="PSUM"))
    ltpool = ctx.enter_context(tc.tile_pool(name="ltpool", bufs=2))
    spool = ctx.enter_context(tc.tile_pool(name="spool", bufs=4))
    xt_pool = ctx.enter_context(tc.tile_pool(name="xt", bufs=2))

    warm = singles.tile([P, P], BF16)
    warm_rhs = singles.tile([P, 4 * P], BF16)
    nc.vector.memset(warm[:], 0.0)
    nc.vector.memset(warm_rhs[:], 0.0)

    warm_flip = [False]

    def warm_mm():
        warm_flip[0] = not warm_flip[0]
        if warm_flip[0]:
            pw = psum_w.tile([P, 4 * P], F32, tag="warm_ps")
        else:
            pw = psum_f.tile([P, 4 * P], F32, tag="fin_ps")
        nc.tensor.matmul(pw[:], warm[:], warm_rhs[:])

    for _ in range(N_WARM):
        warm_mm()

    xf32 = {}
    xb = [
        singles.tile([P, DIM], BF16, tag=f"xb{t}", name=f"xb{t}") for t in range(NB)
    ]
    for t in range(NB):
        if t not in S_TILES:
            xf32[t] = singles.tile([P, DIM], F32, tag=f"xf{t}", name=f"xf{t}")

    ident = singles.tile([P, P], BF16)
    ident_f = singles.tile([P, P], F32)
    wgT = singles.tile([P, KT, E], BF16)

    def x_src(t):
        return x_ap[t * P : (t + 1) * P, :]

    # constants + SWDGE tiles on the gpsimd ring (no DMAHW lanes consumed)
    nc.gpsimd.dma_start(out=ident[:], in_=idb_ap)
    nc.gpsimd.dma_start(out=ident_f[:], in_=idf_ap)
    nc.gpsimd.dma_start(out=wgT[:], in_=wgt_ap)
    for t in S_TILES:
        nc.gpsimd.dma_start(out=xb[t][:], in_=x_src(t))
    # 8 up-front HWDGE DMAs == the 8 DMAHW lanes, so no dispatch waits
    for t in SYNC_TILES[:4]:
        nc.sync.dma_start(out=xf32[t][:], in_=x_src(t))
    for t in ACT_TILES[:4]:
        nc.scalar.dma_start(out=xf32[t][:], in_=x_src(t))

    dd_all = singles.tile([P, NB, 2], F32)
    sig_scratch = singles.tile([1, 2], F32)
    sig_preloaded = [False]

    def cast_tile(t):
        # fp32 -> bf16 token-major, split across engines
        xf = xf32[t]
        if t in GP_CAST:
            nc.vector.tensor_copy(xb[t][:, 0:768], xf[:, 0:768])
            nc.scalar.copy(xb[t][:, 768:1536], xf[:, 768:1536])
            nc.gpsimd.tensor_copy(xb[t][:, 1536:DIM], xf[:, 1536:DIM])
        else:
            nc.vector.tensor_copy(xb[t][:, 0:1024], xf[:, 0:1024])
            nc.scalar.copy(xb[t][:, 1024:DIM], xf[:, 1024:DIM])
        if not sig_preloaded[0]:
            # preload the sigmoid ACT table while the stream runs
            sig_preloaded[0] = True
            nc.scalar.activation(
                sig_scratch[:], ident_f[0:1, 0:2], mybir.ActivationFunctionType.Sigmoid
            )

    drain_ct = [0]

    def transpose_tile(t, xt_g):
        # 16 regular bf16 matmuls vs identity; xt_g layout [P, tb, k, 128t]
        # so drains are contiguous and the main matmul reads k-slices strided
        tb = t % TPG
        for q in range(KT // 4):
            pt = psum_t.tile([P, 4 * P], F32)
            for j in range(4):
                k = 4 * q + j
                nc.tensor.matmul(
                    pt[:, j * P : (j + 1) * P],
                    xb[t][:, k * P : (k + 1) * P],
                    ident[:],
                )
            dst = xt_g[:, tb, 4 * q : 4 * q + 4, :]
            # ~28/64 drains on DVE, rest on ACT (budget balance)
            drain_ct[0] += 1
            if drain_ct[0] % 16 < 7:
                nc.vector.tensor_copy(dst, pt[:])
            else:
                nc.scalar.copy(dst, pt[:])

    def epilogue(lp, width, tiles):
        # back to token-major + top-2 for `width` tokens
        lt = ltpool.tile([E, G], F32, tag="lt")
        nc.vector.tensor_copy(lt[:, 0:width], lp[:, 0:width])
        for i, t in enumerate(tiles):
            c = i * P
            fp = psum_f.tile([P, E], F32, tag="fin_ps")
            nc.tensor.matmul(
                fp[:], lt[:, c : c + P], ident_f[:E, :E], is_transpose=True
            )
            max8 = spool.tile([P, 8], F32)
            nc.vector.max(out=max8[:], in_=fp[:])
            nc.vector.tensor_sub(dd_all[:, t, 0:1], max8[:, 0:1], max8[:, 1:2])
            nc.vector.tensor_sub(dd_all[:, t, 1:2], max8[:, 1:2], max8[:, 0:1])

    late_by_trigger = {}
    for late_t, trig in LATE_EMIT.items():
        late_by_trigger.setdefault(trig, []).append(late_t)

    def maybe_emit_late(t):
        for late_t in late_by_trigger.get(t, []):
            eng = nc.sync if late_t in SYNC_TILES else nc.scalar
            eng.dma_start(out=xf32[late_t][:], in_=x_src(late_t))

    for g in range(NG):
        xt_g = xt_pool.tile([P, TPG, KT, P], BF16)
        last = g == NG - 1
        for tb in range(TPG):
            t = g * TPG + tb
            if t not in S_TILES:
                cast_tile(t)
            transpose_tile(t, xt_g)
            maybe_emit_late(t)
            if tb % 2 == 1 and not last:
                warm_mm()  # keeper: bridge PE data-waits below HAM's MID window
            if last and tb == 1:
                # pair {12,13} main (N=256) while 14/15 still stream
                lp = psum_l.tile([E, G], F32, tag="lp")
                for k in range(KT):
                    nc.tensor.matmul(
                        lp[:, 0 : 2 * P],
                        wgT[:, k, :],
                        xt_g[:, 0:2, k, :],
                        start=(k == 0),
                        stop=(k == KT - 1),
                    )
                epilogue(lp, 2 * P, [g * TPG, g * TPG + 1])
            if last and tb >= 2:
                # per-tile mains (N=128) for 14 and 15: shortest serial tail
                lp = psum_l.tile([E, G], F32, tag="lp")
                for k in range(KT):
                    nc.tensor.matmul(
                        lp[:, 0:P],
                        wgT[:, k, :],
                        xt_g[:, tb, k, :],
                        start=(k == 0),
                        stop=(k == KT - 1),
                    )
                epilogue(lp, P, [t])
        if not last:
            lp = psum_l.tile([E, G], F32, tag="lp")
            for k in range(KT):
                nc.tensor.matmul(
                    lp[:],
                    wgT[:, k, :],
                    xt_g[:, :, k, :],
                    start=(k == 0),
                    stop=(k == KT - 1),
                )
            epilogue(lp, G, [g * TPG + i for i in range(TPG)])

    # single sigmoid + one contiguous partition-major store
    ot = singles.tile([P, NB, 2], F32)
    nc.scalar.activation(ot[:], dd_all[:], mybir.ActivationFunctionType.Sigmoid)
    nc.sync.dma_start(out=out_ap, in_=ot[:])


_NC_CACHE = {}


def _build():
    key = "nc"
    if key in _NC_CACHE:
        return _NC_CACHE[key]
    nc = bacc.Bacc(trn_type="TRN2")
    x = nc.dram_tensor("x", [TPC, DIM], F32, kind="ExternalInput")
    wgt = nc.dram_tensor("wgt", [P, KT * E], BF16, kind="ExternalInput")
    idb = nc.dram_tensor("idb", [P, P], BF16, kind="ExternalInput")
    idf = nc.dram_tensor("idf", [P, P], F32, kind="ExternalInput")
    out = nc.dram_tensor("out", [P, NB * 2], F32, kind="ExternalOutput")
    with TileContext(nc) as tc, ExitStack() as ctx:
        _emit(tc, ctx, x.ap(), wgt.ap(), idb.ap(), idf.ap(), out.ap())
    if not nc.is_finalized():
        nc.finalize()
    _NC_CACHE[key] = nc
    return nc


def _run(x, W_g, trace=False):
    nc = _build()
    x = np.ascontiguousarray(np.asarray(x, dtype=np.float32))
    W_g = np.asarray(W_g, dtype=np.float32)
    # host-side weight layout prep: wgt[p, k*E + e] = W_g[e, k*128 + p]
    wgt = np.ascontiguousarray(
        W_g.reshape(E, KT, P).transpose(2, 1, 0).reshape(P, KT * E)
    ).astype(ml_dtypes.bfloat16)
    idb = np.eye(P, dtype=np.float32).astype(ml_dtypes.bfloat16)
    idf = np.eye(P, dtype=np.float32)
    in_maps = [
        {
            "x": np.ascontiguousarray(x[c * TPC : (c + 1) * TPC]),
            "wgt": wgt,
            "idb": idb,
            "idf": idf,
        }
        for c in range(NCORES)
    ]
    res = run_bass_kernel_spmd(nc, in_maps, core_ids=list(range(NCORES)), trace=trace)
    # device output is partition-major [128, 16, 2]; de-interleave:
    # out[b*128 + p, :] = res[p, b, :]
    outs = []
    for r in res.results:
        o = r["out"].reshape(P, NB, 2).transpose(1, 0, 2).reshape(TPC, 2)
        outs.append(o)
    out = np.ascontiguousarray(np.concatenate(outs, axis=0))
    return out, res


def kernel(x, W_g):
    out, _ = _run(x, W_g, trace=False)
    return out


def kernel_profiled(x, W_g, **_kw):
    out, res = _run(x, W_g, trace=True)
    return out, res


# revision 7
# speedup vs baseline: 1.1570x; 1.1076x over previous
"""MoE gating kernel for Trainium2 (Bass/Tile), data-parallel over 8 NeuronCores.

Computes: logits = x @ W_g.T ; top-2 values; softmax over the 2 values.
  p1 = sigmoid(v1 - v2), p2 = sigmoid(v2 - v1)  (v1 >= v2 the top-2 logits)

Sharding: tokens split 8 ways (2048 tokens/core), W_g replicated.

v4 design (from the 73.2us baseline):
  - The x stream is engine-bound: the 16 SDMA engines process one 8KB fp32
    row per ~306ns (~27 GB/s each) regardless of DGE path; 2048 rows take
    ~39.2us aggregate. Engines round-robin the ACTIVE queues fairly at
    packet granularity, so a ring's tiles arrive at n_rings * 2.45us
    cadence, in ring-FIFO order.
  - 8 tiles ride HWDGE fp32 (4 on sync + 4 on scalar ring == the 8 DMAHW
    completion-sem lanes, so no dispatch waits ever block a sequencer
    FIFO; that cascade serialized earlier attempts to 100us+). These are
    cast fp32->bf16 on-chip, ONE engine per tile (alternating DVE/ACT) so
    each tile's transposes depend on a single cast op.
  - 8 tiles + the constants ride the SWDGE converting path (bf16 lands
    directly). Tile numbering interleaves rings [sync,act,pool,...] early
    so arrival order matches the in-order pipeline; the last 4 tiles are
    all pool: once the HWDGE rings drain (~35us) the pool ring gets full
    engine bandwidth and delivers the tail at 2.45us cadence.
  - bf16 transposes via regular matmul against identity (HAM-visible),
    G=512 grouped main matmuls; final group split per-pair then per-tile
    so the serial tail after the last byte is one tile, not a group.
  - Sigmoid ACT table preloaded early; one batched sigmoid + one
    contiguous partition-major store at the end; host de-interleaves.
"""

import sys

sys.path.insert(0, "/opt/trn_rl_repo")

from contextlib import ExitStack

import numpy as np
import ml_dtypes

import concourse.bass as bass
import concourse.bacc as bacc
import concourse.mybir as mybir
from concourse.tile import TileContext
from concourse.bass_utils import run_bass_kernel_spmd

TOKENS = 16384
DIM = 2048
E = 64  # num experts
NCORES = 8
TPC = TOKENS // NCORES  # tokens per core
P = 128
KT = DIM // P  # 16 contraction tiles
NB = TPC // P  # 16 token blocks (tiles) per core
G = 512  # token group for the main matmul
TPG = G // P  # 4 tiles per group
NG = TPC // G  # 4 groups

F32 = mybir.dt.float32
BF16 = mybir.dt.bfloat16
N_WARM = 20

SYNC_TILES = (0, 3, 6, 9)  # HWDGE sync ring, ring order
ACT_TILES = (1, 4, 7, 10)  # HWDGE scalar ring, ring order
S_TILES = (2, 5, 8, 11, 12, 13, 14, 15)  # SWDGE pool ring, ring order
CAST_DVE = frozenset({0, 3, 6, 9})  # cast engine per HWDGE tile
# drain engine per (tile, quad): cross-assigned from the cast engine
def drain_on_dve(t, q):
    if t in CAST_DVE:
        return False  # ACT casts... no: t cast on DVE -> drains on ACT
    if t in ACT_TILES:
        return True
    # S tiles: mix ~half/half with slight ACT bias
    return q == 0 or (q == 2 and t % 2 == 0)


def _emit(tc, ctx, x_ap, wgt_ap, idb_ap, idf_ap, out_ap):
    nc = tc.nc

    singles = ctx.enter_context(tc.tile_pool(name="singles", bufs=1))
    psum_t = ctx.enter_context(tc.tile_pool(name="psum_t", bufs=3, space="PSUM"))
    psum_l = ctx.enter_context(tc.tile_pool(name="psum_l", bufs=2, space="PSUM"))
    psum_f = ctx.enter_context(tc.tile_pool(name="psum_f", bufs=2, space="PSUM"))
    psum_w = ctx.enter_context(tc.tile_pool(name="psum_w", bufs=1, space="PSUM"))
    ltpool = ctx.enter_context(tc.tile_pool(name="ltpool", bufs=2))
    spool = ctx.enter_context(tc.tile_pool(name="spool", bufs=4))
    xt_pool = ctx.enter_context(tc.tile_pool(name="xt", bufs=2))

    warm = singles.tile([P, P], BF16)
    warm_rhs = singles.tile([P, 4 * P], BF16)
    nc.vector.memset(warm[:], 0.0)
    nc.vector.memset(warm_rhs[:], 0.0)

    warm_flip = [False]

    def warm_mm():
        warm_flip[0] = not warm_flip[0]
        if warm_flip[0]:
            pw = psum_w.tile([P, 4 * P], F32, tag="warm_ps")
        else:
            pw = psum_f.tile([P, 4 * P], F32, tag="fin_ps")
        nc.tensor.matmul(pw[:], warm[:], warm_rhs[:])

    for _ in range(N_WARM):
        warm_mm()

    xf32 = {}
    xb = [
        singles.tile([P, DIM], BF16, tag=f"xb{t}", name=f"xb{t}") for t in range(NB)
    ]
    for t in SYNC_TILES + ACT_TILES:
        xf32[t] = singles.tile([P, DIM], F32, tag=f"xf{t}", name=f"xf{t}")

    ident = singles.tile([P, P], BF16)
    ident_f = singles.tile([P, P], F32)
    wgT = singles.tile([P, KT, E], BF16)

    def x_src(t):
        return x_ap[t * P : (t + 1) * P, :]

    # constants + SWDGE tiles on the gpsimd ring (no DMAHW lanes consumed)
    nc.gpsimd.dma_start(out=ident[:], in_=idb_ap)
    nc.gpsimd.dma_start(out=ident_f[:], in_=idf_ap)
    nc.gpsimd.dma_start(out=wgT[:], in_=wgt_ap)
    for t in S_TILES:
        nc.gpsimd.dma_start(out=xb[t][:], in_=x_src(t))
    for t in SYNC_TILES:
        nc.sync.dma_start(out=xf32[t][:], in_=x_src(t))
    for t in ACT_TILES:
        nc.scalar.dma_start(out=xf32[t][:], in_=x_src(t))

    dd_all = singles.tile([P, NB, 2], F32)
    sig_scratch = singles.tile([1, 2], F32)
    sig_preloaded = [False]

    def cast_tile(t):
        # fp32 -> bf16 token-major on ONE engine (single dep for transposes)
        if t in CAST_DVE:
            nc.vector.tensor_copy(xb[t][:], xf32[t][:])
        else:
            nc.scalar.copy(xb[t][:], xf32[t][:])
        if not sig_preloaded[0]:
            # preload the sigmoid ACT table while the stream runs
            sig_preloaded[0] = True
            nc.scalar.activation(
                sig_scratch[:], ident_f[0:1, 0:2], mybir.ActivationFunctionType.Sigmoid
            )

    def transpose_tile(t, xt_g):
        # 16 regular bf16 matmuls vs identity; xt_g layout [P, tb, k, 128t]
        # so drains are contiguous and the main matmul reads k-slices strided
        tb = t % TPG
        for q in range(KT // 4):
            pt = psum_t.tile([P, 4 * P], F32)
            for j in range(4):
                k = 4 * q + j
                nc.tensor.matmul(
                    pt[:, j * P : (j + 1) * P],
                    xb[t][:, k * P : (k + 1) * P],
                    ident[:],
                )
            dst = xt_g[:, tb, 4 * q : 4 * q + 4, :]
            if drain_on_dve(t, q):
                nc.vector.tensor_copy(dst, pt[:])
            else:
                nc.scalar.copy(dst, pt[:])

    def epilogue(lp, width, tiles):
        # back to token-major + top-2 for `width` tokens
        lt = ltpool.tile([E, G], F32, tag="lt")
        nc.scalar.copy(lt[:, 0:width], lp[:, 0:width])
        for i, t in enumerate(tiles):
            c = i * P
            fp = psum_f.tile([P, E], F32, tag="fin_ps")
            nc.tensor.matmul(
                fp[:], lt[:, c : c + P], ident_f[:E, :E], is_transpose=True
            )
            max8 = spool.tile([P, 8], F32)
            nc.vector.max(out=max8[:], in_=fp[:])
            nc.vector.tensor_sub(dd_all[:, t, 0:1], max8[:, 0:1], max8[:, 1:2])
            nc.vector.tensor_sub(dd_all[:, t, 1:2], max8[:, 1:2], max8[:, 0:1])

    for g in range(NG):
        xt_g = xt_pool.tile([P, TPG, KT, P], BF16)
        last = g == NG - 1
        for tb in range(TPG):
            t = g * TPG + tb
            if t in xf32:
                cast_tile(t)
            transpose_tile(t, xt_g)
            if tb % 2 == 1 and not last:
                warm_mm()  # keeper: bridge PE data-waits below HAM's MID window
            if last and tb == 1:
                # pair {12,13} main (N=256) while 14/15 still stream
                lp = psum_l.tile([E, G], F32, tag="lp")
                for k in range(KT):
                    nc.tensor.matmul(
                        lp[:, 0 : 2 * P],
                        wgT[:, k, :],
                        xt_g[:, 0:2, k, :],
                        start=(k == 0),
                        stop=(k == KT - 1),
                    )
                epilogue(lp, 2 * P, [g * TPG, g * TPG + 1])
            if last and tb >= 2:
                # per-tile mains (N=128) for 14 and 15: shortest serial tail
                lp = psum_l.tile([E, G], F32, tag="lp")
                for k in range(KT):
                    nc.tensor.matmul(
                        lp[:, 0:P],
                        wgT[:, k, :],
                        xt_g[:, tb, k, :],
                        start=(k == 0),
                        stop=(k == KT - 1),
                    )
                epilogue(lp, P, [t])
        if not last:
            lp = psum_l.tile([E, G], F32, tag="lp")
            for k in range(KT):
                nc.tensor.matmul(
                    lp[:],
                    wgT[:, k, :],
                    xt_g[:, :, k, :],
                    start=(k == 0),
                    stop=(k == KT - 1),
                )
            epilogue(lp, G, [g * TPG + i for i in range(TPG)])

    # single sigmoid + one contiguous partition-major store
    ot = singles.tile([P, NB, 2], F32)
    nc.scalar.activation(ot[:], dd_all[:], mybir.ActivationFunctionType.Sigmoid)
    nc.sync.dma_start(out=out_ap, in_=ot[:])


_NC_CACHE = {}


def _build():
    key = "nc"
    if key in _NC_CACHE:
        return _NC_CACHE[key]
    nc = bacc.Bacc(trn_type="TRN2")
    x = nc.dram_tensor("x", [TPC, DIM], F32, kind="ExternalInput")
    wgt = nc.dram_tensor("wgt", [P, KT * E], BF16, kind="ExternalInput")
    idb = nc.dram_tensor("idb", [P, P], BF16, kind="ExternalInput")
    idf = nc.dram_tensor("idf", [P, P], F32, kind="ExternalInput")
    out = nc.dram_tensor("out", [P, NB * 2], F32, kind="ExternalOutput")
    with TileContext(nc) as tc, ExitStack() as ctx:
        _emit(tc, ctx, x.ap(), wgt.ap(), idb.ap(), idf.ap(), out.ap())
    if not nc.is_finalized():
        nc.finalize()
    _NC_CACHE[key] = nc
    return nc


def _run(x, W_g, trace=False):
    nc = _build()
    x = np.ascontiguousarray(np.asarray(x, dtype=np.float32))
    W_g = np.asarray(W_g, dtype=np.float32)
    # host-side weight layout prep: wgt[p, k*E + e] = W_g[e, k*128 + p]
    wgt = np.ascontiguousarray(
        W_g.reshape(E, KT, P).transpose(2, 1, 0).reshape(P, KT * E)
    ).astype(ml_dtypes.bfloat16)
    idb = np.eye(P, dtype=np.float32).astype(ml_dtypes.bfloat16)
    idf = np.eye(P, dtype=np.float32)
    in_maps = [
        {
            "x": np.ascontiguousarray(x[c * TPC : (c + 1) * TPC]),
            "wgt": wgt,
            "idb": idb,
            "idf": idf,
        }
        for c in range(NCORES)
    ]
    res = run_bass_kernel_spmd(nc, in_maps, core_ids=list(range(NCORES)), trace=trace)
    # device output is partition-major [128, 16, 2]; de-interleave:
    # out[b*128 + p, :] = res[p, b, :]
    outs = []
    for r in res.results:
        o = r["out"].reshape(P, NB, 2).transpose(1, 0, 2).reshape(TPC, 2)
        outs.append(o)
    out = np.ascontiguousarray(np.concatenate(outs, axis=0))
    return out, res


def kernel(x, W_g):
    out, _ = _run(x, W_g, trace=False)
    return out


def kernel_profiled(x, W_g, **_kw):
    out, res = _run(x, W_g, trace=True)
    return out, res


# revision 8
# speedup vs baseline: 1.1722x; 1.0131x over previous
"""MoE gating kernel for Trainium2 (Bass/Tile), data-parallel over 8 NeuronCores.

Computes: logits = x @ W_g.T ; top-2 values; softmax over the 2 values.
  p1 = sigmoid(v1 - v2), p2 = sigmoid(v2 - v1)  (v1 >= v2 the top-2 logits)

Sharding: tokens split 8 ways (2048 tokens/core), W_g replicated.

v5 design (baseline pipeline + restructured stream; baseline was 73.2us):
  - Stream: the 16 SDMA engines are the bottleneck (~27GB/s each, one 8KB
    fp32 row per ~306ns; 2048 rows = ~39.2us aggregate). Engines
    round-robin active queues fairly, so per-ring tile cadence is
    n_active_rings * 2.45us. The baseline put everything on the SWDGE
    pool ring: robust in-order 2.45us arrivals, but the Q7 boot delays
    the first descriptor to ~9.4us and engines 7/15 straggle ~7% on
    SWDGE rows (trailing tile ~60us).
  - Here 8 tiles ride HWDGE fp32 (4 sync + 4 scalar == the 8 DMAHW
    completion-sem lanes; a 9th+ HWDGE DMA would stall the issuing
    sequencer's whole FIFO on a lane-reuse wait — that cascade cost two
    previous attempts 30us). HWDGE dispatch starts ~6.8us, ~3us before
    the Q7's first descriptor. The other 8 tiles + constants ride SWDGE
    converting DMA. Tile numbering interleaves [sync,act,pool] early so
    arrival order tracks the in-order pipeline; the last 4 tiles are all
    pool: once the HWDGE rings drain (~35us) the pool ring gets full
    engine bandwidth for the tail.
  - HWDGE fp32 tiles are cast to bf16 by ONE engine each (alternating
    DVE/ACT) so each tile's transposes have a single cast dependency.
  - Everything else keeps the baseline's proven choreography: bf16
    transposes as regular matmuls vs identity (HAM-visible, warm-up +
    keeper matmuls flip and hold the PE clock gate), G=256 groups,
    3:5 DVE/ACT PSUM drains, epilogue lagging one group, except:
      * the final group runs per-tile mains + immediate epilogue so the
        serial tail after the last byte is one tile, not a group;
      * the sigmoid ACT table is preloaded early off the critical path;
      * psum_f is double-buffered so fin transposes don't ping-pong with
        max8.
  - One batched sigmoid + one contiguous partition-major store; host
    de-interleaves.
"""

import sys

sys.path.insert(0, "/opt/trn_rl_repo")

from contextlib import ExitStack

import numpy as np
import ml_dtypes

import concourse.bass as bass
import concourse.bacc as bacc
import concourse.mybir as mybir
from concourse.tile import TileContext
from concourse.bass_utils import run_bass_kernel_spmd

TOKENS = 16384
DIM = 2048
E = 64  # num experts
NCORES = 8
TPC = TOKENS // NCORES  # tokens per core
P = 128
KT = DIM // P  # 16 contraction tiles
G = 256  # token group (moving-dim of the big matmul)
NG = TPC // G  # 8 groups per core
TB = G // P  # 2 token blocks per group
NB = NG * TB  # 16 token blocks per core

F32 = mybir.dt.float32
BF16 = mybir.dt.bfloat16
N_WARM = 20

SYNC_TILES = (0, 3, 6, 9)  # HWDGE sync ring, ring order
ACT_TILES = (1, 4, 7, 10)  # HWDGE scalar ring, ring order
S_TILES = (2, 5, 8, 11, 12, 13, 14, 15)  # SWDGE pool ring, ring order
CAST_DVE = frozenset({0, 3, 6, 9})  # whole-tile cast engine per HWDGE tile


def _emit(tc, ctx, x_ap, wgt_ap, idb_ap, idf_ap, out_ap):
    nc = tc.nc

    singles = ctx.enter_context(tc.tile_pool(name="singles", bufs=1))
    xtpool = ctx.enter_context(tc.tile_pool(name="xtpool", bufs=3))
    ltpool = ctx.enter_context(tc.tile_pool(name="ltpool", bufs=2))
    spool = ctx.enter_context(tc.tile_pool(name="spool", bufs=4))
    psum_t = ctx.enter_context(tc.tile_pool(name="psum_t", bufs=3, space="PSUM"))
    psum_l = ctx.enter_context(tc.tile_pool(name="psum_l", bufs=2, space="PSUM"))
    psum_f = ctx.enter_context(tc.tile_pool(name="psum_f", bufs=2, space="PSUM"))
    psum_w = ctx.enter_context(tc.tile_pool(name="psum_w", bufs=1, space="PSUM"))

    warm = singles.tile([P, P], BF16)
    warm_rhs = singles.tile([P, 4 * P], BF16)
    nc.vector.memset(warm[:], 0.0)
    nc.vector.memset(warm_rhs[:], 0.0)

    warm_flip = [False]

    def warm_mm():
        # alternate PSUM banks: back-to-back matmuls into ONE bank
        # serialize on the write-after-write; alternating sustains the
        # ~80% duty HAM needs to flip
        warm_flip[0] = not warm_flip[0]
        if warm_flip[0]:
            pw = psum_w.tile([P, 4 * P], F32, tag="warm_ps")
        else:
            pw = psum_f.tile([P, 4 * P], F32, tag="fin_ps")
        nc.tensor.matmul(pw[:], warm[:], warm_rhs[:])

    for _ in range(N_WARM):
        warm_mm()

    def keeper(n=1):
        for _ in range(n):
            warm_mm()

    xf32 = {}
    xb = [
        singles.tile([P, DIM], BF16, tag=f"xb{t}", name=f"xb{t}") for t in range(NB)
    ]
    for t in SYNC_TILES + ACT_TILES:
        xf32[t] = singles.tile([P, DIM], F32, tag=f"xf{t}", name=f"xf{t}")

    ident = singles.tile([P, P], BF16)
    ident_f = singles.tile([P, P], F32)
    wgT = singles.tile([P, KT, E], BF16)

    def x_src(t):
        return x_ap[t * P : (t + 1) * P, :]

    # constants + converting tile DMAs on the SWDGE pool ring
    nc.gpsimd.dma_start(out=ident[:], in_=idb_ap)
    nc.gpsimd.dma_start(out=ident_f[:], in_=idf_ap)
    nc.gpsimd.dma_start(out=wgT[:], in_=wgt_ap)
    for t in S_TILES:
        nc.gpsimd.dma_start(out=xb[t][:], in_=x_src(t))
    for t in SYNC_TILES:
        nc.sync.dma_start(out=xf32[t][:], in_=x_src(t))
    for t in ACT_TILES:
        nc.scalar.dma_start(out=xf32[t][:], in_=x_src(t))

    # per-token-block v1-v2 / v2-v1 accumulate here; one sigmoid + one
    # contiguous partition-major store at the end
    dd_all = singles.tile([P, NB, 2], F32)
    sig_scratch = singles.tile([1, 2], F32)
    sig_preloaded = [False]

    def cast_tile(t):
        if t in CAST_DVE:
            nc.vector.tensor_copy(xb[t][:], xf32[t][:])
        else:
            nc.scalar.copy(xb[t][:], xf32[t][:])
        if not sig_preloaded[0]:
            sig_preloaded[0] = True
            nc.scalar.activation(
                sig_scratch[:], ident_f[0:1, 0:2], mybir.ActivationFunctionType.Sigmoid
            )

    def epilogue(g, lp, tiles=None):
        # back to token-major + top-2 (normally runs one group late)
        tiles = tiles if tiles is not None else [g * TB + tb for tb in range(TB)]
        lt = ltpool.tile([E, G], F32)
        for i, t in enumerate(tiles):
            nc.vector.tensor_copy(
                lt[:, i * P : (i + 1) * P], lp[:, i * P : (i + 1) * P]
            )
            fp = psum_f.tile([P, E], F32, tag="fin_ps")
            nc.tensor.matmul(
                fp[:],
                lt[:, i * P : (i + 1) * P],
                ident_f[:E, :E],
                is_transpose=True,
            )
            max8 = spool.tile([P, 8], F32)
            nc.vector.max(out=max8[:], in_=fp[:])
            nc.vector.tensor_sub(dd_all[:, t, 0:1], max8[:, 0:1], max8[:, 1:2])
            nc.vector.tensor_sub(dd_all[:, t, 1:2], max8[:, 1:2], max8[:, 0:1])

    pending = None  # (g, lp) awaiting epilogue
    for g in range(NG):
        tiles = [g * TB + tb for tb in range(TB)]
        for t in tiles:
            if t in xf32:
                cast_tile(t)

        if g >= 1:
            keeper(1 if g < 4 else 2)

        # transpose into xT via regular bf16 matmuls against the identity
        xt = xtpool.tile([P, KT * G], BF16)
        for q in range(KT // 2):
            pt = psum_t.tile([P, 2 * G], F32)
            for dk in range(2):
                k = 2 * q + dk
                for tb in range(TB):
                    nc.tensor.matmul(
                        pt[:, dk * G + tb * P : dk * G + (tb + 1) * P],
                        xb[tiles[tb]][:, k * P : (k + 1) * P],
                        ident[:],
                    )
            dst = xt[:, 2 * q * G : (2 * q + 2) * G]
            if q < 3:
                nc.vector.tensor_copy(dst, pt[:])
            else:
                nc.scalar.copy(dst, pt[:])

        if g < NG - 1:
            # logitsT [64 e, 256 t] = sum_k wgT_k.T @ xT_k
            lp = psum_l.tile([E, G], F32)
            for k in range(KT):
                nc.tensor.matmul(
                    lp[:],
                    wgT[:, k, :],
                    xt[:, k * G : (k + 1) * G],
                    start=(k == 0),
                    stop=(k == KT - 1),
                )
            if pending is not None:
                epilogue(*pending)
            pending = (g, lp)
        else:
            # final group: per-tile mains + immediate epilogue so the
            # serial tail after the last tile's arrival is minimal
            if pending is not None:
                epilogue(*pending)
                pending = None
            for tb in range(TB):
                lp = psum_l.tile([E, G], F32)
                for k in range(KT):
                    nc.tensor.matmul(
                        lp[:, 0:P],
                        wgT[:, k, :],
                        xt[:, k * G + tb * P : k * G + (tb + 1) * P],
                        start=(k == 0),
                        stop=(k == KT - 1),
                    )
                epilogue(g, lp, tiles=[g * TB + tb])
    if pending is not None:
        epilogue(*pending)

    # single sigmoid + one contiguous partition-major store
    ot = singles.tile([P, NB, 2], F32)
    nc.scalar.activation(ot[:], dd_all[:], mybir.ActivationFunctionType.Sigmoid)
    nc.sync.dma_start(out=out_ap, in_=ot[:])


_NC_CACHE = {}


def _build():
    key = "nc"
    if key in _NC_CACHE:
        return _NC_CACHE[key]
    nc = bacc.Bacc(trn_type="TRN2")
    x = nc.dram_tensor("x", [TPC, DIM], F32, kind="ExternalInput")
    wgt = nc.dram_tensor("wgt", [P, KT * E], BF16, kind="ExternalInput")
    idb = nc.dram_tensor("idb", [P, P], BF16, kind="ExternalInput")
    idf = nc.dram_tensor("idf", [P, P], F32, kind="ExternalInput")
    out = nc.dram_tensor("out", [P, NB * 2], F32, kind="ExternalOutput")
    with TileContext(nc) as tc, ExitStack() as ctx:
        _emit(tc, ctx, x.ap(), wgt.ap(), idb.ap(), idf.ap(), out.ap())
    if not nc.is_finalized():
        nc.finalize()
    _NC_CACHE[key] = nc
    return nc


def _run(x, W_g, trace=False):
    nc = _build()
    x = np.ascontiguousarray(np.asarray(x, dtype=np.float32))
    W_g = np.asarray(W_g, dtype=np.float32)
    # host-side weight layout prep: wgt[p, k*E + e] = W_g[e, k*128 + p]
    wgt = np.ascontiguousarray(
        W_g.reshape(E, KT, P).transpose(2, 1, 0).reshape(P, KT * E)
    ).astype(ml_dtypes.bfloat16)
    idb = np.eye(P, dtype=np.float32).astype(ml_dtypes.bfloat16)
    idf = np.eye(P, dtype=np.float32)
    in_maps = [
        {
            "x": np.ascontiguousarray(x[c * TPC : (c + 1) * TPC]),
            "wgt": wgt,
            "idb": idb,
            "idf": idf,
        }
        for c in range(NCORES)
    ]
    res = run_bass_kernel_spmd(nc, in_maps, core_ids=list(range(NCORES)), trace=trace)
    # device output is partition-major [128, 16, 2]; de-interleave:
    # out[b*128 + p, :] = res[p, b, :]
    outs = []
    for r in res.results:
        o = r["out"].reshape(P, NB, 2).transpose(1, 0, 2).reshape(TPC, 2)
        outs.append(o)
    out = np.ascontiguousarray(np.concatenate(outs, axis=0))
    return out, res


def kernel(x, W_g):
    out, _ = _run(x, W_g, trace=False)
    return out


def kernel_profiled(x, W_g, **_kw):
    out, res = _run(x, W_g, trace=True)
    return out, res


# revision 9
# speedup vs baseline: 1.4129x; 1.2053x over previous
"""MoE gating kernel for Trainium2 (Bass/Tile), data-parallel over 8 NeuronCores.

Computes: logits = x @ W_g.T ; top-2 values; softmax over the 2 values.
  p1 = sigmoid(v1 - v2), p2 = sigmoid(v2 - v1)  (v1 >= v2 the top-2 logits)

Sharding: tokens split 8 ways (2048 tokens/core), W_g replicated.

v6 design (from the 73.2us baseline):
  - The x stream is engine-bound: the 16 SDMA engines process one 8KB fp32
    row per ~306ns (~27 GB/s each); 2048 rows = ~39.2us aggregate,
    regardless of DGE path. Engines round-robin ACTIVE queues fairly, so
    total stream time is conserved; only ordering and start time are
    controllable. Two hard-won constraints shape this design:
      * every engine FIFO executes its program in a fixed order, so an
        instruction gated on a late tile arrival blocks every ready
        instruction behind it on that engine (this cost three multi-ring
        attempts 20-30us each);
      * the Tile scheduler has 8 DMAHW completion lanes for HWDGE DMAs;
        a 9th in-flight HWDGE DMA stalls the issuing sequencer's FIFO.
  - So: the SWDGE pool ring (baseline's proven in-order 2.45us/tile
    cadence) carries 12 tiles; HWDGE carries 4 "early" tiles that cover
    the pool ring's two weak spots: the Q7 boot (~9.4us before its first
    descriptor; HWDGE dispatches at ~6.8us) and the tail. The four HWDGE
    tiles all ARRIVE early (~11-15us) and are PROCESSED early: the
    process order is [0, 1, 14, 15, 2, 3, ..., 13] so process order ==
    arrival order and no FIFO ever blocks. The host unpermutes at the
    end (free).
  - The 4 HWDGE fp32 tiles are cast to bf16 whole-tile (2 on DVE, 2 on
    ACT); they sit at the head of those FIFOs where nothing is queued
    behind, so their arrival-gating is harmless.
  - Everything else is the baseline's proven choreography: bf16
    transposes as regular matmuls vs identity (HAM-visible; warm-up +
    keeper matmuls hold the PE clock gate), G=256 groups, 3:5 DVE/ACT
    PSUM drains, epilogue lagging one group — except the final processed
    group {12,13} runs per-tile mains + immediate epilogue, the sigmoid
    ACT table is preloaded early, and psum_f is double-buffered.
  - One batched sigmoid + one contiguous partition-major store.
"""

import sys

sys.path.insert(0, "/opt/trn_rl_repo")

from contextlib import ExitStack

import numpy as np
import ml_dtypes

import concourse.bass as bass
import concourse.bacc as bacc
import concourse.mybir as mybir
from concourse.tile import TileContext
from concourse.bass_utils import run_bass_kernel_spmd

TOKENS = 16384
DIM = 2048
E = 64  # num experts
NCORES = 8
TPC = TOKENS // NCORES  # tokens per core
P = 128
KT = DIM // P  # 16 contraction tiles
G = 256  # token group (moving-dim of the big matmul)
NG = TPC // G  # 8 groups per core
TB = G // P  # 2 token blocks per group
NB = NG * TB  # 16 token blocks per core

F32 = mybir.dt.float32
BF16 = mybir.dt.bfloat16
N_WARM = 20

SYNC_TILES = (0, 14)  # HWDGE sync ring, ring order
ACT_TILES = (1, 15)  # HWDGE scalar ring, ring order
S_TILES = tuple(range(2, 14))  # SWDGE pool ring, ring order
CAST_DVE = frozenset({0, 14})  # whole-tile cast engine per HWDGE tile
# process order == arrival order; host unpermutes
PROC_ORDER = (0, 1, 14, 15) + tuple(range(2, 14))


def _emit(tc, ctx, x_ap, wgt_ap, idb_ap, idf_ap, out_ap):
    nc = tc.nc

    singles = ctx.enter_context(tc.tile_pool(name="singles", bufs=1))
    xtpool = ctx.enter_context(tc.tile_pool(name="xtpool", bufs=3))
    ltpool = ctx.enter_context(tc.tile_pool(name="ltpool", bufs=2))
    spool = ctx.enter_context(tc.tile_pool(name="spool", bufs=4))
    psum_t = ctx.enter_context(tc.tile_pool(name="psum_t", bufs=3, space="PSUM"))
    psum_l = ctx.enter_context(tc.tile_pool(name="psum_l", bufs=2, space="PSUM"))
    psum_f = ctx.enter_context(tc.tile_pool(name="psum_f", bufs=2, space="PSUM"))
    psum_w = ctx.enter_context(tc.tile_pool(name="psum_w", bufs=1, space="PSUM"))

    warm = singles.tile([P, P], BF16)
    warm_rhs = singles.tile([P, 4 * P], BF16)
    nc.vector.memset(warm[:], 0.0)
    nc.vector.memset(warm_rhs[:], 0.0)

    warm_flip = [False]

    def warm_mm():
        # alternate PSUM banks: back-to-back matmuls into ONE bank
        # serialize on the write-after-write; alternating sustains the
        # ~80% duty HAM needs to flip
        warm_flip[0] = not warm_flip[0]
        if warm_flip[0]:
            pw = psum_w.tile([P, 4 * P], F32, tag="warm_ps")
        else:
            pw = psum_f.tile([P, 4 * P], F32, tag="fin_ps")
        nc.tensor.matmul(pw[:], warm[:], warm_rhs[:])

    for _ in range(N_WARM):
        warm_mm()

    def keeper(n=1):
        for _ in range(n):
            warm_mm()

    xf32 = {}
    xb = [
        singles.tile([P, DIM], BF16, tag=f"xb{t}", name=f"xb{t}") for t in range(NB)
    ]
    for t in SYNC_TILES + ACT_TILES:
        xf32[t] = singles.tile([P, DIM], F32, tag=f"xf{t}", name=f"xf{t}")

    ident = singles.tile([P, P], BF16)
    ident_f = singles.tile([P, P], F32)
    wgT = singles.tile([P, KT, E], BF16)

    def x_src(t):
        return x_ap[t * P : (t + 1) * P, :]

    # constants then converting tile DMAs on the SWDGE pool ring, in
    # process order; HWDGE head/early tiles on sync+scalar (4 DMAs + the
    # out store stay within the 8 DMAHW lanes: no dispatch waits)
    nc.gpsimd.dma_start(out=ident[:], in_=idb_ap)
    nc.gpsimd.dma_start(out=wgT[:], in_=wgt_ap)
    nc.gpsimd.dma_start(out=ident_f[:], in_=idf_ap)
    for t in S_TILES:
        nc.gpsimd.dma_start(out=xb[t][:], in_=x_src(t))
    for t in SYNC_TILES:
        nc.sync.dma_start(out=xf32[t][:], in_=x_src(t))
    for t in ACT_TILES:
        nc.scalar.dma_start(out=xf32[t][:], in_=x_src(t))

    # per-process-position (v1-v2, v2-v1) accumulate here; one sigmoid +
    # one contiguous partition-major store at the end
    dd_all = singles.tile([P, NB, 2], F32)
    sig_scratch = singles.tile([1, 2], F32)
    sig_preloaded = [False]

    def cast_tile(t):
        if t in CAST_DVE:
            nc.vector.tensor_copy(xb[t][:], xf32[t][:])
        else:
            nc.scalar.copy(xb[t][:], xf32[t][:])
        if not sig_preloaded[0]:
            sig_preloaded[0] = True
            nc.scalar.activation(
                sig_scratch[:], ident_f[0:1, 0:2], mybir.ActivationFunctionType.Sigmoid
            )

    def epilogue(g, lp, positions=None):
        # back to token-major + top-2 (normally runs one group late);
        # `positions` are process positions (dd_all slots)
        positions = positions if positions is not None else [g * TB + i for i in range(TB)]
        lt = ltpool.tile([E, G], F32)
        for i, b in enumerate(positions):
            nc.vector.tensor_copy(
                lt[:, i * P : (i + 1) * P], lp[:, i * P : (i + 1) * P]
            )
            fp = psum_f.tile([P, E], F32, tag="fin_ps")
            nc.tensor.matmul(
                fp[:],
                lt[:, i * P : (i + 1) * P],
                ident_f[:E, :E],
                is_transpose=True,
            )
            max8 = spool.tile([P, 8], F32)
            nc.vector.max(out=max8[:], in_=fp[:])
            nc.vector.tensor_sub(dd_all[:, b, 0:1], max8[:, 0:1], max8[:, 1:2])
            nc.vector.tensor_sub(dd_all[:, b, 1:2], max8[:, 1:2], max8[:, 0:1])

    pending = None  # (g, lp) awaiting epilogue
    for g in range(NG):
        tiles = [PROC_ORDER[g * TB + i] for i in range(TB)]
        for t in tiles:
            if t in xf32:
                cast_tile(t)

        if g >= 1:
            keeper(1 if g < 4 else 2)

        # transpose into xT via regular bf16 matmuls against the identity
        xt = xtpool.tile([P, KT * G], BF16)
        for q in range(KT // 2):
            pt = psum_t.tile([P, 2 * G], F32)
            for dk in range(2):
                k = 2 * q + dk
                for tb in range(TB):
                    nc.tensor.matmul(
                        pt[:, dk * G + tb * P : dk * G + (tb + 1) * P],
                        xb[tiles[tb]][:, k * P : (k + 1) * P],
                        ident[:],
                    )
            dst = xt[:, 2 * q * G : (2 * q + 2) * G]
            if q < 3:
                nc.vector.tensor_copy(dst, pt[:])
            else:
                nc.scalar.copy(dst, pt[:])

        if g < NG - 1:
            # logitsT [64 e, 256 t] = sum_k wgT_k.T @ xT_k
            lp = psum_l.tile([E, G], F32)
            for k in range(KT):
                nc.tensor.matmul(
                    lp[:],
                    wgT[:, k, :],
                    xt[:, k * G : (k + 1) * G],
                    start=(k == 0),
                    stop=(k == KT - 1),
                )
            if pending is not None:
                epilogue(*pending)
            pending = (g, lp)
        else:
            # final processed group: per-tile mains + immediate epilogue
            # so the serial tail after the last arrival is minimal
            if pending is not None:
                epilogue(*pending)
                pending = None
            for tb in range(TB):
                lp = psum_l.tile([E, G], F32)
                for k in range(KT):
                    nc.tensor.matmul(
                        lp[:, 0:P],
                        wgT[:, k, :],
                        xt[:, k * G + tb * P : k * G + (tb + 1) * P],
                        start=(k == 0),
                        stop=(k == KT - 1),
                    )
                epilogue(g, lp, positions=[g * TB + tb])
    if pending is not None:
        epilogue(*pending)

    # single sigmoid + one contiguous partition-major store
    ot = singles.tile([P, NB, 2], F32)
    nc.scalar.activation(ot[:], dd_all[:], mybir.ActivationFunctionType.Sigmoid)
    nc.sync.dma_start(out=out_ap, in_=ot[:])


_NC_CACHE = {}


def _build():
    key = "nc"
    if key in _NC_CACHE:
        return _NC_CACHE[key]
    nc = bacc.Bacc(trn_type="TRN2")
    x = nc.dram_tensor("x", [TPC, DIM], F32, kind="ExternalInput")
    wgt = nc.dram_tensor("wgt", [P, KT * E], BF16, kind="ExternalInput")
    idb = nc.dram_tensor("idb", [P, P], BF16, kind="ExternalInput")
    idf = nc.dram_tensor("idf", [P, P], F32, kind="ExternalInput")
    out = nc.dram_tensor("out", [P, NB * 2], F32, kind="ExternalOutput")
    with TileContext(nc) as tc, ExitStack() as ctx:
        _emit(tc, ctx, x.ap(), wgt.ap(), idb.ap(), idf.ap(), out.ap())
    if not nc.is_finalized():
        nc.finalize()
    _NC_CACHE[key] = nc
    return nc


def _run(x, W_g, trace=False):
    nc = _build()
    x = np.ascontiguousarray(np.asarray(x, dtype=np.float32))
    W_g = np.asarray(W_g, dtype=np.float32)
    # host-side weight layout prep: wgt[p, k*E + e] = W_g[e, k*128 + p]
    wgt = np.ascontiguousarray(
        W_g.reshape(E, KT, P).transpose(2, 1, 0).reshape(P, KT * E)
    ).astype(ml_dtypes.bfloat16)
    idb = np.eye(P, dtype=np.float32).astype(ml_dtypes.bfloat16)
    idf = np.eye(P, dtype=np.float32)
    in_maps = [
        {
            "x": np.ascontiguousarray(x[c * TPC : (c + 1) * TPC]),
            "wgt": wgt,
            "idb": idb,
            "idf": idf,
        }
        for c in range(NCORES)
    ]
    res = run_bass_kernel_spmd(nc, in_maps, core_ids=list(range(NCORES)), trace=trace)
    # device output is partition-major [128, 16, 2] in PROCESS order;
    # de-interleave + unpermute: out[PROC_ORDER[b]*128 + p] = res[p, b]
    inv = np.argsort(np.array(PROC_ORDER))
    outs = []
    for r in res.results:
        o = r["out"].reshape(P, NB, 2)[:, inv, :].transpose(1, 0, 2).reshape(TPC, 2)
        outs.append(o)
    out = np.ascontiguousarray(np.concatenate(outs, axis=0))
    return out, res


def kernel(x, W_g):
    out, _ = _run(x, W_g, trace=False)
    return out


def kernel_profiled(x, W_g, **_kw):
    out, res = _run(x, W_g, trace=True)
    return out, res


# revision 10
# speedup vs baseline: 1.6817x; 1.1903x over previous
"""MoE gating kernel for Trainium2 (Bass/Tile), data-parallel over 8 NeuronCores.

Computes: logits = x @ W_g.T ; top-2 values; softmax over the 2 values.
  p1 = sigmoid(v1 - v2), p2 = sigmoid(v2 - v1)  (v1 >= v2 the top-2 logits)

Sharding: tokens split 8 ways (2048 tokens/core), W_g replicated.

v7 design (from the 73.2us baseline; v6 measured 74.5us PE-bound):
  - Stream: the 16 SDMA engines process one 8KB fp32 row per ~306ns each;
    2048 rows = ~39.2us aggregate on any DGE path. Engines round-robin
    active queues fairly, so only ordering/start are controllable.
    Constraints learned the hard way: an engine FIFO executes in fixed
    order, so an op gated on a late arrival blocks everything behind it
    (multi-ring cast schemes lost 20-30us to this); HWDGE has 8 DMAHW
    completion lanes, a 9th in-flight DMA stalls the issuing sequencer.
  - So: the SWDGE pool ring carries 12 tiles in process order (in-order
    2.45us/tile cadence once solo); HWDGE carries 4 early tiles covering
    the Q7 boot gap, processed in arrival order [0, 1, 14, 15, 2..13]
    (host unpermutes). Their fp32->bf16 casts are whole-tile, 2 on DVE /
    2 on ACT, at FIFO positions matching their arrival.
  - Compute is fully per-tile (no group barrier): 16 bf16 transposes vs
    identity (regular matmuls: HAM-visible), drains split DVE/ACT, then
    16 x-STATIONARY mains: out[t,e] += xT_k.T @ wgT_k with the xT slice
    as the 128-col stationary (FWL) and the tiny wgT as moving — logits
    land token-major [128t, 64e] in PSUM, so max8 reads PSUM directly.
    This kills the logitsT->token-major fin transposes, the lt copies,
    and shortens the serial tail to one tile's chain.
  - Warm-up + keeper matmuls hold the PE clock gate (HAM) through the
    early arrival gaps; sigmoid ACT table preloaded early; one batched
    sigmoid + one contiguous partition-major store; host de-interleaves.
"""

import sys

sys.path.insert(0, "/opt/trn_rl_repo")

from contextlib import ExitStack

import numpy as np
import ml_dtypes

import concourse.bass as bass
import concourse.bacc as bacc
import concourse.mybir as mybir
from concourse.tile import TileContext
from concourse.bass_utils import run_bass_kernel_spmd

TOKENS = 16384
DIM = 2048
E = 64  # num experts
NCORES = 8
TPC = TOKENS // NCORES  # tokens per core
P = 128
KT = DIM // P  # 16 contraction tiles
NB = TPC // P  # 16 token blocks (tiles) per core

F32 = mybir.dt.float32
BF16 = mybir.dt.bfloat16
N_WARM = 20

SYNC_TILES = (0, 14)  # HWDGE sync ring, ring order
ACT_TILES = (1, 15)  # HWDGE scalar ring, ring order
S_TILES = tuple(range(2, 14))  # SWDGE pool ring, ring order
CAST_DVE = frozenset({0, 14})  # whole-tile cast engine per HWDGE tile
# process order == arrival order; host unpermutes
PROC_ORDER = (0, 1, 14, 15) + tuple(range(2, 14))


def _emit(tc, ctx, x_ap, wgt_ap, idb_ap, idf_ap, out_ap):
    nc = tc.nc

    singles = ctx.enter_context(tc.tile_pool(name="singles", bufs=1))
    xtpool = ctx.enter_context(tc.tile_pool(name="xtpool", bufs=4))
    spool = ctx.enter_context(tc.tile_pool(name="spool", bufs=4))
    psum_t = ctx.enter_context(tc.tile_pool(name="psum_t", bufs=3, space="PSUM"))
    psum_l = ctx.enter_context(tc.tile_pool(name="psum_l", bufs=2, space="PSUM"))
    psum_f = ctx.enter_context(tc.tile_pool(name="psum_f", bufs=1, space="PSUM"))
    psum_w = ctx.enter_context(tc.tile_pool(name="psum_w", bufs=1, space="PSUM"))

    warm = singles.tile([P, P], BF16)
    warm_rhs = singles.tile([P, 4 * P], BF16)
    nc.vector.memset(warm[:], 0.0)
    nc.vector.memset(warm_rhs[:], 0.0)

    warm_flip = [False]

    def warm_mm():
        # alternate PSUM banks: back-to-back matmuls into ONE bank
        # serialize on the write-after-write; alternating sustains the
        # ~80% duty HAM needs to flip
        warm_flip[0] = not warm_flip[0]
        if warm_flip[0]:
            pw = psum_w.tile([P, 4 * P], F32, tag="warm_ps")
        else:
            pw = psum_f.tile([P, 4 * P], F32, tag="fin_ps")
        nc.tensor.matmul(pw[:], warm[:], warm_rhs[:])

    for _ in range(N_WARM):
        warm_mm()

    def keeper(n=1):
        for _ in range(n):
            warm_mm()

    xf32 = {}
    xb = [
        singles.tile([P, DIM], BF16, tag=f"xb{t}", name=f"xb{t}") for t in range(NB)
    ]
    for t in SYNC_TILES + ACT_TILES:
        xf32[t] = singles.tile([P, DIM], F32, tag=f"xf{t}", name=f"xf{t}")

    ident = singles.tile([P, P], BF16)
    ident_f = singles.tile([P, P], F32)
    wgT = singles.tile([P, KT, E], BF16)

    def x_src(t):
        return x_ap[t * P : (t + 1) * P, :]

    # constants then converting tile DMAs on the SWDGE pool ring, in
    # process order; HWDGE tiles on sync+scalar (4 DMAs + the out store
    # stay within the 8 DMAHW lanes: no dispatch waits)
    nc.gpsimd.dma_start(out=ident[:], in_=idb_ap)
    nc.gpsimd.dma_start(out=wgT[:], in_=wgt_ap)
    nc.gpsimd.dma_start(out=ident_f[:], in_=idf_ap)
    for t in S_TILES:
        nc.gpsimd.dma_start(out=xb[t][:], in_=x_src(t))
    for t in SYNC_TILES:
        nc.sync.dma_start(out=xf32[t][:], in_=x_src(t))
    for t in ACT_TILES:
        nc.scalar.dma_start(out=xf32[t][:], in_=x_src(t))

    # per-process-position (v1-v2, v2-v1) accumulate here; one sigmoid +
    # one contiguous partition-major store at the end
    dd_all = singles.tile([P, NB, 2], F32)
    sig_scratch = singles.tile([1, 2], F32)
    sig_preloaded = [False]

    def cast_tile(t):
        if t in CAST_DVE:
            nc.vector.tensor_copy(xb[t][:], xf32[t][:])
        else:
            nc.scalar.copy(xb[t][:], xf32[t][:])
        if not sig_preloaded[0]:
            sig_preloaded[0] = True
            nc.scalar.activation(
                sig_scratch[:], ident_f[0:1, 0:2], mybir.ActivationFunctionType.Sigmoid
            )

    for pos in range(NB):
        t = PROC_ORDER[pos]
        if t in xf32:
            cast_tile(t)
        if pos >= 2 and pos % 2 == 0:
            keeper(1 if pos < 8 else 2)

        # 16 regular bf16 transposes vs identity -> xt_t [128d-slices, t]
        xt_t = xtpool.tile([P, KT * P], BF16)
        for q in range(KT // 4):
            pt = psum_t.tile([P, 4 * P], F32)
            for j in range(4):
                k = 4 * q + j
                nc.tensor.matmul(
                    pt[:, j * P : (j + 1) * P],
                    xb[t][:, k * P : (k + 1) * P],
                    ident[:],
                )
            dst = xt_t[:, 4 * q * P : (4 * q + 4) * P]
            if q % 2 == 0:
                nc.vector.tensor_copy(dst, pt[:])
            else:
                nc.scalar.copy(dst, pt[:])

        # x-stationary mains: logits land token-major [128t, 64e] in PSUM
        fp = psum_l.tile([P, E], F32)
        for k in range(KT):
            nc.tensor.matmul(
                fp[:],
                xt_t[:, k * P : (k + 1) * P],
                wgT[:, k, :],
                start=(k == 0),
                stop=(k == KT - 1),
            )
        max8 = spool.tile([P, 8], F32)
        nc.vector.max(out=max8[:], in_=fp[:])
        nc.vector.tensor_sub(dd_all[:, pos, 0:1], max8[:, 0:1], max8[:, 1:2])
        nc.vector.tensor_sub(dd_all[:, pos, 1:2], max8[:, 1:2], max8[:, 0:1])

    # single sigmoid + one contiguous partition-major store
    ot = singles.tile([P, NB, 2], F32)
    nc.scalar.activation(ot[:], dd_all[:], mybir.ActivationFunctionType.Sigmoid)
    nc.sync.dma_start(out=out_ap, in_=ot[:])


_NC_CACHE = {}


def _build():
    key = "nc"
    if key in _NC_CACHE:
        return _NC_CACHE[key]
    nc = bacc.Bacc(trn_type="TRN2")
    x = nc.dram_tensor("x", [TPC, DIM], F32, kind="ExternalInput")
    wgt = nc.dram_tensor("wgt", [P, KT * E], BF16, kind="ExternalInput")
    idb = nc.dram_tensor("idb", [P, P], BF16, kind="ExternalInput")
    idf = nc.dram_tensor("idf", [P, P], F32, kind="ExternalInput")
    out = nc.dram_tensor("out", [P, NB * 2], F32, kind="ExternalOutput")
    with TileContext(nc) as tc, ExitStack() as ctx:
        _emit(tc, ctx, x.ap(), wgt.ap(), idb.ap(), idf.ap(), out.ap())
    if not nc.is_finalized():
        nc.finalize()
    _NC_CACHE[key] = nc
    return nc


def _run(x, W_g, trace=False):
    nc = _build()
    x = np.ascontiguousarray(np.asarray(x, dtype=np.float32))
    W_g = np.asarray(W_g, dtype=np.float32)
    # host-side weight layout prep: wgt[p, k*E + e] = W_g[e, k*128 + p]
    wgt = np.ascontiguousarray(
        W_g.reshape(E, KT, P).transpose(2, 1, 0).reshape(P, KT * E)
    ).astype(ml_dtypes.bfloat16)
    idb = np.eye(P, dtype=np.float32).astype(ml_dtypes.bfloat16)
    idf = np.eye(P, dtype=np.float32)
    in_maps = [
        {
            "x": np.ascontiguousarray(x[c * TPC : (c + 1) * TPC]),
            "wgt": wgt,
            "idb": idb,
            "idf": idf,
        }
        for c in range(NCORES)
    ]
    res = run_bass_kernel_spmd(nc, in_maps, core_ids=list(range(NCORES)), trace=trace)
    # device output is partition-major [128, 16, 2] in PROCESS order;
    # de-interleave + unpermute: out[PROC_ORDER[b]*128 + p] = res[p, b]
    inv = np.argsort(np.array(PROC_ORDER))
    outs = []
    for r in res.results:
        o = r["out"].reshape(P, NB, 2)[:, inv, :].transpose(1, 0, 2).reshape(TPC, 2)
        outs.append(o)
    out = np.ascontiguousarray(np.concatenate(outs, axis=0))
    return out, res


def kernel(x, W_g):
    out, _ = _run(x, W_g, trace=False)
    return out


def kernel_profiled(x, W_g, **_kw):
    out, res = _run(x, W_g, trace=True)
    return out, res
